# revision 1
# baseline (speedup 1.0000x reference)
"""Trainium2 Bass kernel for ContrastiveMaskedPatchSimilarity loss.

Computes: per-position cosine similarity along the channel axis of two
[32, 256, 64, 64] f32 tensors, then a masked mean -> scalar.

Strategy (pure data parallel over 8 NeuronCores, batch-sharded 4 each):
  - Layout on chip: [channel-chunk (128) = partitions, spatial (4096) = free].
    DMA of u/m tiles is perfectly contiguous per partition (16KB rows).
  - Elementwise products (u*m, u*u, m*m) on DVE/ACT, written as bf16.
  - Channel reduction via TensorE: product slice [128ch x 128pos] is the
    *stationary* operand (lhsT), rhs = ones[128,1] bf16 -> out[128pos, 1]
    lands position-major in PSUM, so the epilogue runs with all 128
    partitions busy.
  - Epilogue per batch: num/(sqrt(uu*mm)), fused multiply+reduce with the
    (host-pretransposed) mask, free-axis reduction -> [128, 8] partials.
  - Host: sum partials over cores, divide.
"""

import sys
from contextlib import ExitStack

import numpy as np

sys.path.insert(0, "/opt/trn_rl_repo")

import ml_dtypes  # noqa: E402

import concourse.bass as bass  # noqa: E402
import concourse.tile as tile  # noqa: E402
from concourse import bacc, mybir  # noqa: E402
from concourse.bass_utils import run_bass_kernel_spmd  # noqa: E402

B, C, H, W = 32, 256, 64, 64
NCORES = 8
BL = B // NCORES  # batches per core: 4
HWX = H * W  # 4096
ROWS = BL * C  # 1024
NPB = HWX // 128  # position blocks per batch: 32
NCHUNK = C // 128  # channel chunks: 2

F32 = mybir.dt.float32
BF16 = mybir.dt.bfloat16

_CACHED_NC = None


def build_nc():
    nc = bacc.Bacc(
        "TRN2", target_bir_lowering=False, debug=False, num_devices=NCORES
    )
    u_d = nc.dram_tensor("u", [ROWS, HWX], F32, kind="ExternalInput")
    m_d = nc.dram_tensor("m", [ROWS, HWX], F32, kind="ExternalInput")
    # mask, pre-transposed on host to [p_in (128), b*NPB + pb (128)] f32
    mk_d = nc.dram_tensor("maskf", [128, BL * NPB], F32, kind="ExternalInput")
    ones_d = nc.dram_tensor("ones", [128, 1], BF16, kind="ExternalInput")
    # out[:, 0:BL] = per-batch sum(sim*mask) partials (per partition)
    # out[:, BL:2BL] = per-batch sum(mask) partials (per partition)
    out_d = nc.dram_tensor("out", [128, 2 * BL], F32, kind="ExternalOutput")

    with tile.TileContext(nc) as tc, ExitStack() as ctx:
        const_pool = ctx.enter_context(tc.tile_pool(name="const", bufs=1))
        in_pool = ctx.enter_context(tc.tile_pool(name="inp", bufs=6))
        tmp_pool = ctx.enter_context(tc.tile_pool(name="tmp", bufs=3))
        ep_pool = ctx.enter_context(tc.tile_pool(name="ep", bufs=2))
        acc_pool = ctx.enter_context(tc.tile_pool(name="acc", bufs=1))
        psum_pool = ctx.enter_context(
            tc.tile_pool(name="psum", bufs=2, space="PSUM")
        )

        ones_t = const_pool.tile([128, 1], BF16)
        nc.sync.dma_start(ones_t[:], ones_d[:, :])
        maskf_t = const_pool.tile([128, BL * NPB], F32)
        nc.sync.dma_start(maskf_t[:], mk_d[:, :])
        acc_t = acc_pool.tile([128, 2 * BL], F32)
        # mask-only sums don't depend on tensor data: do them up front
        for b in range(BL):
            nc.vector.tensor_reduce(
                acc_t[:, BL + b : BL + b + 1],
                maskf_t[:, b * NPB : (b + 1) * NPB],
                axis=mybir.AxisListType.X,
                op=mybir.AluOpType.add,
            )

        HHX = HWX // 2  # half-tile free dim (1MB DMAs, earlier pipeline ramp)
        HPB = HHX // 128  # position blocks per half: 16
        mm_ctr = 0
        for b in range(BL):
            # PSUM cols: ch*3*NPB + stat*NPB + (h*HPB + pb)
            P = psum_pool.tile([128, NCHUNK * 3 * NPB], F32)
            for ch in range(NCHUNK):
                row0 = b * C + ch * 128
                for h in range(2):
                    csl = slice(h * HHX, (h + 1) * HHX)
                    u_t = in_pool.tile([128, HHX], F32, tag="u")
                    nc.sync.dma_start(u_t[:], u_d[row0 : row0 + 128, csl])
                    m_t = in_pool.tile([128, HHX], F32, tag="m")
                    nc.gpsimd.dma_start(m_t[:], m_d[row0 : row0 + 128, csl])

                    num_t = tmp_pool.tile([128, HHX], BF16, tag="num")
                    nc.vector.tensor_mul(num_t[:], u_t[:], m_t[:])
                    uu_t = tmp_pool.tile([128, HHX], BF16, tag="uu")
                    nc.scalar.square(uu_t[:], u_t[:])
                    mm_t = tmp_pool.tile([128, HHX], BF16, tag="mm")
                    # balance m*m between DVE (faster) and ACT so neither
                    # engine exceeds the DMA roofline
                    if mm_ctr % 3 == 0:
                        nc.vector.tensor_mul(mm_t[:], m_t[:], m_t[:])
                    else:
                        nc.scalar.square(mm_t[:], m_t[:])
                    mm_ctr += 1

                    for s, t in enumerate((num_t, uu_t, mm_t)):
                        base = ch * 3 * NPB + s * NPB + h * HPB
                        for pb in range(HPB):
                            nc.tensor.matmul(
                                P[:, base + pb : base + pb + 1],
                                t[:, pb * 128 : (pb + 1) * 128],
                                ones_t[:, :],
                                start=True,
                                stop=True,
                            )

            # epilogue for batch b (position-major [128, NPB] tiles)
            def psl(ch, s):
                c0 = ch * 3 * NPB + s * NPB
                return P[:, c0 : c0 + NPB]

            # DVE has a single PSUM read port: copy chunk-0 stats to SBUF
            # on ACT first, then add with only one PSUM operand per op.
            n0 = ep_pool.tile([128, NPB], F32, tag="n0")
            nc.scalar.copy(n0[:], psl(0, 0))
            u0 = ep_pool.tile([128, NPB], F32, tag="u0")
            nc.scalar.copy(u0[:], psl(0, 1))
            m0 = ep_pool.tile([128, NPB], F32, tag="m0")
            nc.scalar.copy(m0[:], psl(0, 2))
            numv = ep_pool.tile([128, NPB], F32, tag="numv")
            nc.vector.tensor_add(numv[:], n0[:], psl(1, 0))
            uuv = ep_pool.tile([128, NPB], F32, tag="uuv")
            nc.vector.tensor_add(uuv[:], u0[:], psl(1, 1))
            mmv = ep_pool.tile([128, NPB], F32, tag="mmv")
            nc.vector.tensor_add(mmv[:], m0[:], psl(1, 2))
            d2 = ep_pool.tile([128, NPB], F32, tag="d2")
            nc.vector.tensor_mul(d2[:], uuv[:], mmv[:])
            r = ep_pool.tile([128, NPB], F32, tag="r")
            nc.vector.reciprocal(r[:], d2[:])
            rs = ep_pool.tile([128, NPB], F32, tag="rs")
            nc.scalar.sqrt(rs[:], r[:])
            sim_t = ep_pool.tile([128, NPB], F32, tag="sim")
            nc.vector.tensor_mul(sim_t[:], numv[:], rs[:])
            simmask = ep_pool.tile([128, NPB], F32, tag="simmask")
            nc.vector.tensor_mul(
                simmask[:], sim_t[:], maskf_t[:, b * NPB : (b + 1) * NPB]
            )
            nc.vector.tensor_reduce(
                acc_t[:, b : b + 1],
                simmask[:],
                axis=mybir.AxisListType.X,
                op=mybir.AluOpType.add,
            )

        nc.sync.dma_start(out_d[:, :], acc_t[:])

    nc.compile()
    return nc


def get_nc():
    global _CACHED_NC
    if _CACHED_NC is None:
        _CACHED_NC = build_nc()
    return _CACHED_NC


def make_in_maps(unmasked, masked, latent_mask):
    ones = np.ones((128, 1), dtype=ml_dtypes.bfloat16)
    in_maps = []
    for i in range(NCORES):
        sl = slice(i * BL, (i + 1) * BL)
        u = np.ascontiguousarray(unmasked[sl]).reshape(ROWS, HWX)
        m = np.ascontiguousarray(masked[sl]).reshape(ROWS, HWX)
        mk = (
            latent_mask[sl]
            .reshape(128, 128)
            .T.astype(np.float32)
        )
        in_maps.append(
            {
                "u": u,
                "m": m,
                "maskf": np.ascontiguousarray(mk),
                "ones": ones,
            }
        )
    return in_maps


def _finalize(results):
    num = 0.0
    den = 0.0
    for res in results:
        out = np.asarray(res["out"], dtype=np.float64)
        num += out[:, :BL].sum()
        den += out[:, BL:].sum()
    return np.float32(num / den)


def kernel(unmasked_latent_tensors, masked_latent_tensors, latent_mask, **kw):
    nc = get_nc()
    in_maps = make_in_maps(
        np.asarray(unmasked_latent_tensors, dtype=np.float32),
        np.asarray(masked_latent_tensors, dtype=np.float32),
        np.asarray(latent_mask),
    )
    res = run_bass_kernel_spmd(nc, in_maps, list(range(NCORES)))
    return _finalize(res.results)


def kernel_traced(unmasked_latent_tensors, masked_latent_tensors, latent_mask):
    """Like kernel() but with NTFF tracing; returns (value, BassKernelResults)."""
    nc = get_nc()
    in_maps = make_in_maps(
        np.asarray(unmasked_latent_tensors, dtype=np.float32),
        np.asarray(masked_latent_tensors, dtype=np.float32),
        np.asarray(latent_mask),
    )
    res = run_bass_kernel_spmd(nc, in_maps, list(range(NCORES)), trace=True)
    return _finalize(res.results), res



# revision 10
# speedup vs baseline: 1.3205x; 1.3205x over previous
"""Trainium2 Bass kernel for ContrastiveMaskedPatchSimilarity loss.

Computes: per-position cosine similarity along the channel axis of two
[32, 256, 64, 64] f32 tensors, then a masked mean -> scalar.

Strategy (pure data parallel over 8 NeuronCores, batch-sharded 4 each):
  - Layout on chip: [channel-chunk (128) = partitions, spatial (2048) = free].
    DMA of u/m tiles is contiguous per partition (8KB rows).
  - Both input streams ride HWDGE rings (u on sync/SP, m on scalar/ACT) --
    SWDGE descriptors are ~18% slower per 8KB and eat Q7 time.
  - Work unit = (batch, spatial-half): both 128-channel chunks loaded
    together (4 x 1MB DMAs), six products (u*m, u*u, m*m per chunk) on
    DVE/ACT as bf16.
  - Channel reduction via TensorE: per position-block column, the two
    chunks' product slices [128ch x 128pos] are matmul'd against
    ones[128,1] back-to-back into the same PSUM slot (start/stop
    accumulation) -> no chunk-combining ops anywhere.
  - Per-batch epilogue (emitted one unit late so it never stalls DVE):
    numm=num*mask (off critical path), d2=uu*mm, recip, sqrt, fused
    tensor_tensor_reduce(numm*rs) -> acc[:, b].
  - Host: sum partials over cores, divide by host-computed mask count.
"""

import sys
from contextlib import ExitStack

import numpy as np

sys.path.insert(0, "/opt/trn_rl_repo")

import ml_dtypes  # noqa: E402
import concourse.bass as bass  # noqa: E402
import concourse.tile as tile  # noqa: E402
from concourse import bacc, mybir  # noqa: E402
from concourse.alu_op_type import AluOpType  # noqa: E402
from concourse.bass_utils import run_bass_kernel_spmd  # noqa: E402

B, C, H, W = 32, 256, 64, 64
NCORES = 8
BL = B // NCORES  # batches per core: 4
HWX = H * W  # 4096
ROWS = BL * C  # 1024
NPB = HWX // 128  # position blocks per batch: 32
NCHUNK = C // 128  # channel chunks: 2

F32 = mybir.dt.float32
BF16 = mybir.dt.bfloat16

HHX = HWX // 2  # half-tile free dim (1MB DMAs)
HPB = HHX // 128  # position blocks per half: 16
PREFETCH = 2  # DMA prefetch depth (units issued ahead of compute)

_CACHED_NC = None


def build_nc():
    nc = bacc.Bacc(
        "TRN2", target_bir_lowering=False, debug=False, num_devices=NCORES
    )
    u_d = nc.dram_tensor("u", [ROWS, HWX], F32, kind="ExternalInput")
    m_d = nc.dram_tensor("m", [ROWS, HWX], F32, kind="ExternalInput")
    # mask, pre-transposed on host to [p_in (128), b*NPB + pb (128)] f32
    mk_d = nc.dram_tensor("maskf", [128, BL * NPB], F32, kind="ExternalInput")
    # out[:, b] = per-batch sum(sim*mask) partials (per partition)
    out_d = nc.dram_tensor("out", [128, BL], F32, kind="ExternalOutput")

    # work unit (b, h): both channel chunks of one spatial half
    units = [(b, h) for b in range(BL) for h in range(2)]

    with tile.TileContext(nc) as tc, ExitStack() as ctx:
        const_pool = ctx.enter_context(tc.tile_pool(name="const", bufs=1))
        in_pool = ctx.enter_context(tc.tile_pool(name="inp", bufs=3))
        tmp_pool = ctx.enter_context(tc.tile_pool(name="tmp", bufs=2))
        ep_pool = ctx.enter_context(tc.tile_pool(name="ep", bufs=2))
        acc_pool = ctx.enter_context(tc.tile_pool(name="acc", bufs=1))
        psum_pool = ctx.enter_context(
            tc.tile_pool(name="psum", bufs=2, space="PSUM")
        )

        ones_t = const_pool.tile([128, 1], BF16)
        nc.vector.memset(ones_t[:], 1.0)
        maskf_t = const_pool.tile([128, BL * NPB], F32)
        nc.sync.dma_start(maskf_t[:], mk_d[:, :])
        acc_t = acc_pool.tile([128, BL], F32)

        in_tiles = {}

        def issue_dma(i):
            b, h = units[i]
            csl = slice(h * HHX, (h + 1) * HHX)
            ums = []
            for ch in range(NCHUNK):
                row0 = b * C + ch * 128
                u_t = in_pool.tile([128, HHX], F32, tag=f"u{ch}")
                nc.sync.dma_start(u_t[:], u_d[row0 : row0 + 128, csl])
                m_t = in_pool.tile([128, HHX], F32, tag=f"m{ch}")
                nc.sync.dma_start(m_t[:], m_d[row0 : row0 + 128, csl])
                ums.append((u_t, m_t))
            in_tiles[i] = ums

        psums = {}

        def epilogue(b):
            P = psums.pop(b)

            def psl(s):
                return P[:, s * NPB : (s + 1) * NPB]

            # DVE has a single PSUM read port: pull uu to SBUF on ACT
            # first so each DVE op touches at most one PSUM operand.
            # Mask folds into num up front (off the critical chain).
            numm = ep_pool.tile([128, NPB], F32, tag="numm")
            nc.vector.tensor_mul(
                numm[:], psl(0), maskf_t[:, b * NPB : (b + 1) * NPB]
            )
            uu_s = ep_pool.tile([128, NPB], F32, tag="uu_s")
            nc.scalar.copy(uu_s[:], psl(1))
            d2 = ep_pool.tile([128, NPB], F32, tag="d2")
            nc.vector.tensor_mul(d2[:], uu_s[:], psl(2))
            r = ep_pool.tile([128, NPB], F32, tag="r")
            nc.vector.reciprocal(r[:], d2[:])
            rs = ep_pool.tile([128, NPB], F32, tag="rs")
            nc.scalar.sqrt(rs[:], r[:])
            smask = ep_pool.tile([128, NPB], F32, tag="smask")
            nc.vector.tensor_mul(smask[:], numm[:], rs[:])
            nc.vector.tensor_reduce(
                acc_t[:, b : b + 1],
                smask[:],
                axis=mybir.AxisListType.X,
                op=mybir.AluOpType.add,
            )

        for j in range(PREFETCH):
            issue_dma(j)

        mm_ctr = 0
        for i, (b, h) in enumerate(units):
            if i + PREFETCH < len(units):
                issue_dma(i + PREFETCH)
            ums = in_tiles.pop(i)

            if h == 0:
                # PSUM cols: s*NPB + pb_global (chunks accumulate in place)
                psums[b] = psum_pool.tile(
                    [128, 3 * NPB], F32, name="P", tag="P"
                )
            P = psums[b]

            prods = []  # prods[ch] = (num, uu, mm)
            for ch, (u_t, m_t) in enumerate(ums):
                num_t = tmp_pool.tile([128, HHX], BF16, tag=f"num{ch}")
                nc.vector.tensor_mul(num_t[:], u_t[:], m_t[:])
                uu_t = tmp_pool.tile([128, HHX], BF16, tag=f"uu{ch}")
                nc.scalar.square(uu_t[:], u_t[:])
                mm_t = tmp_pool.tile([128, HHX], BF16, tag=f"mm{ch}")
                # balance m*m between DVE and ACT so neither exceeds the
                # DMA roofline
                if mm_ctr % 2 == 0:
                    nc.vector.tensor_mul(mm_t[:], m_t[:], m_t[:])
                else:
                    nc.scalar.square(mm_t[:], m_t[:])
                mm_ctr += 1
                prods.append((num_t, uu_t, mm_t))

            for s in range(3):
                base = s * NPB + h * HPB
                for pb in range(HPB):
                    for ch in range(NCHUNK):
                        nc.tensor.matmul(
                            P[:, base + pb : base + pb + 1],
                            prods[ch][s][:, pb * 128 : (pb + 1) * 128],
                            ones_t[:, :],
                            start=(ch == 0),
                            stop=(ch == NCHUNK - 1),
                        )

            # emit epilogue for batch b-1 one unit after its last matmul
            # was emitted, so its PSUM fan-in never stalls the DVE stream
            if h == 0 and b > 0:
                epilogue(b - 1)

        epilogue(BL - 1)
        nc.sync.dma_start(out_d[:, :], acc_t[:])

    nc.compile()
    return nc


def get_nc():
    global _CACHED_NC
    if _CACHED_NC is None:
        _CACHED_NC = build_nc()
    return _CACHED_NC


def make_in_maps(unmasked, masked, latent_mask):
    in_maps = []
    for i in range(NCORES):
        sl = slice(i * BL, (i + 1) * BL)
        u = np.ascontiguousarray(unmasked[sl]).reshape(ROWS, HWX)
        m = np.ascontiguousarray(masked[sl]).reshape(ROWS, HWX)
        mk = (latent_mask[sl] != 0).reshape(128, 128).T.astype(np.float32)
        in_maps.append(
            {"u": u, "m": m, "maskf": np.ascontiguousarray(mk)}
        )
    return in_maps


def _finalize(results, latent_mask):
    num = 0.0
    for res in results:
        num += np.asarray(res["out"], dtype=np.float64).sum()
    den = float((np.asarray(latent_mask) != 0).sum())
    return np.float32(num / den)


def kernel(unmasked_latent_tensors, masked_latent_tensors, latent_mask, **kw):
    nc = get_nc()
    in_maps = make_in_maps(
        np.asarray(unmasked_latent_tensors, dtype=np.float32),
        np.asarray(masked_latent_tensors, dtype=np.float32),
        np.asarray(latent_mask),
    )
    res = run_bass_kernel_spmd(nc, in_maps, list(range(NCORES)))
    return _finalize(res.results, latent_mask)


def kernel_traced(unmasked_latent_tensors, masked_latent_tensors, latent_mask):
    """Like kernel() but with NTFF tracing; returns (value, BassKernelResults)."""
    nc = get_nc()
    in_maps = make_in_maps(
        np.asarray(unmasked_latent_tensors, dtype=np.float32),
        np.asarray(masked_latent_tensors, dtype=np.float32),
        np.asarray(latent_mask),
    )
    res = run_bass_kernel_spmd(nc, in_maps, list(range(NCORES)), trace=True)
    return _finalize(res.results, latent_mask), res
